# revision 26
# baseline (speedup 1.0000x reference)
"""Trainium2 Bass kernel for BoundConvexViolationProjection (Gram-space).

Problem (hardcoded from the reference):
  x [32,8,512] f32, A [32,8,512,512] f32, b [32,8,512] f32, var_mask [32,512] f32 (ones)
  Iterate (up to MAX_ITER=100):
      r    = einsum('bsn,bsmn->bsm', x, A) - b
      viol = relu(r) - relu(-r - DELTA)
      g    = einsum('bsm,bsmn->bsn', viol, A)
      tv   = sum(relu(r), -1);  active = tv >= DELTA
      x    = max(where(active, x - LR*g/(|g|+EPS), x), 0)
  while any(active).  For this problem size all rows stay active for the
  full 100 iterations (verified numerically), so the loop runs exactly
  MAX_ITER times.

Algorithmic transformation (validated vs the f64 reference in numpy):
  The x>=0 clamp binds in only 0.33%% of coordinate-steps and truncates at
  most ~6e-4, so the loop runs UNCLAMPED in residual (M) space:
      r' = r - c * G viol,   G = A A^T   (ONE M x M matvec per iteration
      instead of the A x / A^T viol pair of the direct form -> half the
      PE weight traffic, which is the bottleneck)
      u' += c * viol;        x_fin = relu(x0 - GS * A^T u')
  |g|^2 = viol^T G viol = viol . (G viol) comes for free from the matvec.
  f64 no-clamp error vs reference: 1.2e-4.  Full quantized pipeline
  (fp8 e3m4 G off-diagonal at scale GS=1/16 with the exact f32 diagonal
  D=sum A^2 split out, bf16 moving operands / viol / scratch, bf16 reduce
  terms, bf16 coef broadcast): rel err 1.33e-3 (gate 2e-2).

Sharding: data-parallel over batch B across 8 cores (4 batches = 32 (b,s)
pairs per core); loop state fully local, no collectives.

Per-core kernel:
  Prologue: at (A^T fp8 blocks) DMAs in; PE computes G = Ahat Ahat^T as
  FD=512 fp8 GEMMs (~125us, overlapped with the DMA and the first loop
  steps by the Tile scheduler); DVE quantizes G*GS to fp8 in SBUF with the
  diagonal 128-blocks masked by (1-I)*GS.  r0 = Ahat x0 - b.  Only Ghat'
  (8 MiB fp8) stays SBUF-resident; ar (A rows) prefetches into at's slot
  during the loop for the epilogue.

  Loop, 100 iters x 8 chunks of 4 pairs (steps of 64 fp8 128x128
  LDWEIGHTS+MATMUL FD=1 pairs; with FWL the sustained LDW+MM pair rate is
  ~34ns, and that stream is 99%% of the PE timeline).  All non-stream
  machinery runs at PAIR-of-chunks granularity with a 7-step software
  pipeline so no PE instruction ever waits on fresh DVE/ACT data:
    F(2k)@s, F(2k+1)@s+1:  ws = wq + D'viol, gsq = viol o ws  (DVE burst
        at the stream boundary, into a shared [128,4W] redu tile), then
        GPSIMD partition_all_reduce sums tv|gsq across partitions
    Bhead@burst(s+2/s+3):  fold partials, is_ge gate, issue ACT sqrt
    Btail@burst(s+3/s+4):  reciprocal, coef (bf16), then GPSIMD
        partition_broadcast fans coef out to all partitions - the scale
        tail is a step after the head so the ACT sqrt's ~1us cross-engine
        latency is fully hidden
    A(pair)@burst(s+4/s+5): pair-wide r/u updates + next viol/relur/dv
        (relu on the idle ACT engine; coef read through 0-stride
        to_broadcast APs); viol lands ~2.5 streams before its consumer
        stream(2k)@s+8.
  Moving the reduce/broadcast matmuls to the idle GpSimd engine leaves the
  PE queue a pure uniform LDW+MM stream (33.7ns/block, zero bubbles).  The
  A-section is emitted AFTER the F-section so the in-order DVE queue
  retires the gsq terms at the boundary they become ready.

  Scale folding: coef = (LR/sqrt(GS)) * rsqrt(sum(viol.(GS*w)) + 1e-12)
  serves both the r-update (on ws = GS*w) and the u' accumulation
  (u = GS*u'); the final A^T u' matvec is scaled by -GS against x0.
"""

import numpy as np
import ml_dtypes

import concourse.bacc as bacc
import concourse.bass as bass
import concourse.bass_isa as bass_isa
import concourse.mybir as mybir
import concourse.tile as tile
from concourse.bass_utils import run_bass_kernel_spmd

BF16 = ml_dtypes.bfloat16
F8E3 = ml_dtypes.float8_e3m4

N_CORES = 8
B, S, M, N = 32, 8, 512, 512
B_LOC = B // N_CORES            # 4 batches per core
P = B_LOC * S                   # 32 (b,s) pairs per core
NT = N // 128                   # 4 n-tiles
MT = M // 128                   # 4 m-tiles
LR, DELTA = 0.005, 0.1
N_ITERS = 100
CPP = 4                         # pairs per pipeline chunk
NCH = P // CPP                  # 8 chunks
W = CPP * MT                    # 32 columns per chunk ((mt, jj))
GS = 1.0 / 16.0                 # fp8 off-diagonal Gram scale
C0 = LR / (GS ** 0.5)           # folded step coefficient = 0.02


def _build_nc(n_iters=N_ITERS):
    f32 = mybir.dt.float32
    bf16 = mybir.dt.bfloat16
    f8e3 = mybir.dt.float8e3
    Sqrt = mybir.ActivationFunctionType.Sqrt
    Relu = mybir.ActivationFunctionType.Relu
    Alu = mybir.AluOpType

    nc = bacc.Bacc("TRN2", target_bir_lowering=False)
    at_d = nc.dram_tensor("at", [P, 128, NT, 512], f8e3, kind="ExternalInput")
    ar_d = nc.dram_tensor("arows", [P, 128, MT, 512], f8e3, kind="ExternalInput")
    bt_d = nc.dram_tensor("bt", [128, NCH * W], f32, kind="ExternalInput")
    xt_d = nc.dram_tensor("x0t", [128, NCH * W], f32, kind="ExternalInput")
    dt_d = nc.dram_tensor("dt", [128, NCH * W], f32, kind="ExternalInput")
    im_d = nc.dram_tensor("imask", [128, 128], f32, kind="ExternalInput")
    id_d = nc.dram_tensor("ident", [128, 128], f32, kind="ExternalInput")
    out_d = nc.dram_tensor("xout", [P, 512], f32, kind="ExternalOutput")

    with tile.TileContext(nc) as tc:
        with (
            tc.tile_pool(name="resident", bufs=1) as res_pool,
            tc.tile_pool(name="apool", bufs=1) as a_pool,
            tc.tile_pool(name="glue", bufs=4) as glue_pool,
            tc.tile_pool(name="violp", bufs=NCH + 3) as vb_pool,
            tc.tile_pool(name="dvp", bufs=NCH + 3) as dv_pool,
            tc.tile_pool(name="wsp", bufs=NCH + 3) as ws_pool,
            tc.tile_pool(name="redup", bufs=NCH + 3) as redu_pool,
            tc.tile_pool(name="rstate", bufs=2 * NCH + 2) as r_pool,
            tc.tile_pool(name="ustate", bufs=2 * NCH + 2) as u_pool,
            tc.tile_pool(name="rows", bufs=12) as row_pool,
            tc.tile_pool(name="cbgp", bufs=4) as cbg_pool,
            tc.tile_pool(name="mmps", bufs=4, space=bass.MemorySpace.PSUM) as mm_psum,
            tc.tile_pool(name="finps", bufs=2, space=bass.MemorySpace.PSUM) as fin_psum,
        ):
            # ---- persistent tiles + initial loads ----
            gq_sb = res_pool.tile([128, P, MT, 512], f8e3, tag="gq_sb")
            bt_sb = res_pool.tile([128, NCH * W], f32, tag="bt_sb")
            xt_sb = res_pool.tile([128, NCH * W], f32, tag="xt_sb")
            dt_sb = res_pool.tile([128, NCH * W], f32, tag="dt_sb")
            im_sb = res_pool.tile([128, 128], f32, tag="im_sb")
            id_sb = res_pool.tile([128, 128], f32, tag="id_sb")
            cst = res_pool.tile([128, 1], f32, tag="cst")
            ones1 = res_pool.tile([1, 128], f8e3, tag="ones1")
            nc.vector.memset(cst[:], 1e-12)
            nc.vector.memset(ones1[:], 1.0)

            # init loads via SWDGE (gpsimd), in consumption order
            nc.gpsimd.dma_start(out=bt_sb[:], in_=bt_d[:])
            nc.gpsimd.dma_start(out=xt_sb[:], in_=xt_d[:])
            nc.gpsimd.dma_start(out=dt_sb[:], in_=dt_d[:])
            nc.gpsimd.dma_start(out=im_sb[:], in_=im_d[:])
            nc.gpsimd.dma_start(out=id_sb[:], in_=id_d[:])
            at_sb = a_pool.tile([128, P, NT, 512], f8e3, tag="a")
            for j in range(P):
                nc.gpsimd.dma_start(out=at_sb[:, j], in_=at_d[j])
            x0b = res_pool.tile([128, NCH * W], bf16, tag="x0b")
            nc.vector.tensor_copy(x0b[:], xt_sb[:])

            NP2 = NCH // 2          # chunk pairs: state tiles are pair-wide
            r_cur = [None] * NP2    # [128, 2W] f32
            u_cur = [None] * NP2    # [128, 2W] f32
            violb = [None] * NP2    # [128, 2W] bf16
            dvs = [None] * NP2      # [128, 2W] bf16, (D*GS) o viol
            redus = [None] * NP2    # [128, 4W] bf16, [relur|gsq] x2
            ws_t = [None] * NP2     # [128, 2W] bf16, GS * (G viol)

            def emit_glue_pair(p, r_new):
                """viol/relur/dv for both chunks of pair p, from the pair-wide
                r_new [128, 2W].  One DVE op per quantity; relu on ACT."""
                rc = glue_pool.tile([128, 2 * W], bf16, tag="rc")
                nc.vector.tensor_scalar(out=rc[:], in0=r_new[:], scalar1=0.0,
                                        scalar2=-DELTA, op0=Alu.min, op1=Alu.max)
                vb = vb_pool.tile([128, 2 * W], bf16, tag="vb")
                nc.vector.tensor_tensor(vb[:], r_new[:], rc[:], Alu.subtract)
                redu = redu_pool.tile([128, 4 * W], bf16, tag="redu")
                redus[p] = redu
                # relu on the (mostly idle) ACT engine; strided out AP hits
                # the relur columns of both halves in one op
                nc.scalar.activation(
                    redu[:].rearrange("p (c g x) -> p c g x", c=2, g=2)[:, :, 0, :],
                    r_new[:].rearrange("p (c x) -> p c x", c=2), Relu)
                dv = dv_pool.tile([128, 2 * W], bf16, tag="dv")
                nc.vector.tensor_tensor(
                    dv[:], dt_sb[:, 2 * p * W:(2 * p + 2) * W], vb[:], Alu.mult)
                violb[p], dvs[p] = vb, dv

            # ---- prologue: G = Ahat Ahat^T, quantize, r0 ----
            for c in range(NCH):
                for jj in range(CPP):
                    j = c * CPP + jj
                    for mt in range(MT):
                        gps = mm_psum.tile([128, 512], f32, tag="mm")
                        for nt in range(NT):
                            nc.tensor.matmul(
                                gps[:],
                                at_sb[:, j, nt, mt * 128:(mt + 1) * 128],
                                at_sb[:, j, nt, :],
                                start=(nt == 0), stop=(nt == NT - 1),
                            )
                        nc.vector.tensor_scalar(out=gq_sb[:, j, mt, :],
                                                in0=gps[:], scalar1=GS,
                                                scalar2=None, op0=Alu.mult)
                        nc.vector.tensor_tensor(
                            gq_sb[:, j, mt, mt * 128:(mt + 1) * 128],
                            gps[:, mt * 128:(mt + 1) * 128], im_sb[:], Alu.mult)
                # r0 for chunk c -> half of the pair-wide r tile
                r0ps = mm_psum.tile([128, W], f32, tag="mm")
                for jj in range(CPP):
                    j = c * CPP + jj
                    for mt in range(MT):
                        col = mt * CPP + jj
                        for nt in range(NT):
                            nc.tensor.matmul(
                                r0ps[:, col:col + 1],
                                at_sb[:, j, nt, mt * 128:(mt + 1) * 128],
                                x0b[:, c * W + nt * CPP + jj:
                                     c * W + nt * CPP + jj + 1],
                                start=(nt == 0), stop=(nt == NT - 1),
                            )
                if c % 2 == 0:
                    r_cur[c // 2] = r_pool.tile([128, 2 * W], f32,
                                                tag="r", name="r0pair")
                half = (c % 2) * W
                nc.vector.tensor_tensor(r_cur[c // 2][:, half:half + W],
                                        r0ps[:],
                                        bt_sb[:, c * W:(c + 1) * W], Alu.subtract)
                if c % 2 == 1:
                    emit_glue_pair(c // 2, r_cur[c // 2])

            # epilogue ar prefetch: reuses at's SBUF slot once r0 is done;
            # the DMA lands during the first loop iterations.
            ar_sb = a_pool.tile([128, P, MT, 512], f8e3, tag="a")
            for j in range(P):
                nc.gpsimd.dma_start(out=ar_sb[:, j], in_=ar_d[j])

            # ---- main loop ----
            # Streams run every step (chunk c = s%8); all other machinery at
            # PAIR granularity (chunks 2k,2k+1 share [128,2W] state tiles):
            #   F(2k)@s, F(2k+1)@s+1 -> ws halves + gsq halves of pair redu
            #   MERGED(pair)@end(s+2), Bhead@burst(s+2/s+3),
            #   Btail@burst(s+3/s+4), OUTER(pair)@end(s+4),
            #   A(pair)@burst(s+4/s+5): one [128,2W] op per quantity;
            #   viol ready ~2.5 streams before stream(2k)@s+8.
            pend_A = None        # (pair, cb)
            pend_btail = None    # (pair, sq, mlr)
            pend_bhead = None    # (pair, rowp)
            steps = n_iters * NCH
            for s in range(steps + 4):
                cur = s % NCH if s < steps else None
                pend_A_now, pend_A = pend_A, None
                # B-tail: recip + coef (bf16) for the pair
                if pend_btail is not None:
                    pr, sq, mlr = pend_btail
                    inv = row_pool.tile([1, 2 * CPP], f32, tag="inv")
                    nc.vector.reciprocal(inv[:], sq[:])
                    coef = row_pool.tile([1, 2 * CPP], bf16, tag="coef")
                    nc.vector.tensor_tensor(coef[:], mlr[:], inv[:], Alu.mult)
                    # coef -> all partitions on the idle GpSimd engine
                    # (replaces the rank-1 ones-outer PE matmul)
                    cbg = cbg_pool.tile([128, 2 * CPP], bf16, tag="cbg")
                    nc.gpsimd.partition_broadcast(cbg[:], coef[:])
                    pend_A = (pr, cbg)
                    pend_btail = None
                # B-head: reduce + gate for the pair
                if pend_bhead is not None:
                    pr, rowp = pend_bhead
                    red = row_pool.tile([1, 4 * CPP], f32, tag="red")
                    nc.vector.tensor_reduce(
                        red[:].rearrange("p (c g j) -> p c g j", c=2, g=2),
                        rowp[0:1, :].rearrange("p (c g m j) -> p c g j m",
                                               c=2, g=2, j=CPP),
                        axis=mybir.AxisListType.X, op=Alu.add)
                    redv = red[:].rearrange("p (c x) -> p c x", c=2)
                    mlr = row_pool.tile([1, 2 * CPP], f32, tag="mlr")
                    nc.vector.tensor_scalar(
                        out=mlr[:].rearrange("p (c j) -> p c j", c=2),
                        in0=redv[:, :, 0:CPP], scalar1=DELTA, scalar2=C0,
                        op0=Alu.is_ge, op1=Alu.mult)
                    sq = row_pool.tile([1, 2 * CPP], f32, tag="sq")
                    nc.scalar.activation(
                        sq[:].rearrange("p (c j) -> p c j", c=2),
                        redv[:, :, CPP:2 * CPP], Sqrt, bias=cst[:1, :])
                    pend_btail = (pr, sq, mlr)
                    pend_bhead = None
                # C: the big stream  w_psum = Ghat' viol
                if cur is not None:
                    wq = mm_psum.tile([128, W], f32, tag="mm")
                    vb = violb[cur // 2]
                    vh = (cur % 2) * W
                    for jj in range(CPP):
                        j = cur * CPP + jj
                        for mt in range(MT):
                            col = mt * CPP + jj
                            for kt in range(MT):
                                nc.tensor.matmul(
                                    wq[:, col:col + 1],
                                    gq_sb[:, j, kt, mt * 128:(mt + 1) * 128],
                                    vb[:, vh + kt * CPP + jj:
                                        vh + kt * CPP + jj + 1],
                                    start=(kt == 0), stop=(kt == MT - 1),
                                )
                # F: ws = wq + D'viol ; gsq terms into the pair redu tile
                if cur is not None:
                    pr2 = cur // 2
                    half = (cur % 2) * W
                    if cur % 2 == 0:
                        ws_t[pr2] = ws_pool.tile([128, 2 * W], bf16,
                                                 tag="ws", name="wspair")
                    nc.vector.tensor_tensor(ws_t[pr2][:, half:half + W],
                                            wq[:], dvs[pr2][:, half:half + W],
                                            Alu.add)
                    rhalf = (cur % 2) * 2 * W
                    nc.vector.tensor_tensor(
                        redus[pr2][:, rhalf + W:rhalf + 2 * W],
                        violb[pr2][:, half:half + W],
                        ws_t[pr2][:, half:half + W], Alu.mult)
                    if cur % 2 == 1:
                        # tv|gsq column sums on the idle GpSimd engine
                        # (replaces the ones-reduce PE matmul, one pipeline
                        # stage earlier than MERGED was)
                        rowg = cbg_pool.tile([128, 4 * W], f32, tag="rowg")
                        nc.gpsimd.partition_all_reduce(
                            rowg[:], redus[pr2][:], 128, bass_isa.ReduceOp.add)
                        pend_bhead = (pr2, rowg)
                # A: pair-wide r/u update + next-iter glue
                if pend_A_now is not None:
                    pr, cb = pend_A_now
                    last = (s + 3 >= steps)
                    if not last:
                        t = glue_pool.tile([128, 2 * W], bf16, tag="t")
                        nc.vector.tensor_tensor(
                            t[:].rearrange("p (c m j) -> p c m j", c=2, m=MT),
                            cb[:].rearrange("p (c o j) -> p c o j", c=2, o=1)
                                 .to_broadcast((128, 2, MT, CPP)),
                            ws_t[pr][:].rearrange("p (c m j) -> p c m j",
                                                  c=2, m=MT),
                            Alu.mult)
                        r_new = r_pool.tile([128, 2 * W], f32, tag="r")
                        nc.vector.tensor_tensor(r_new[:], r_cur[pr][:], t[:],
                                                Alu.subtract)
                    ut = glue_pool.tile([128, 2 * W], bf16, tag="ut")
                    nc.vector.tensor_tensor(
                        ut[:].rearrange("p (c m j) -> p c m j", c=2, m=MT),
                        cb[:].rearrange("p (c o j) -> p c o j", c=2, o=1)
                             .to_broadcast((128, 2, MT, CPP)),
                        violb[pr][:].rearrange("p (c m j) -> p c m j",
                                               c=2, m=MT),
                        Alu.mult)
                    u_new = u_pool.tile([128, 2 * W], f32, tag="u")
                    if u_cur[pr] is None:
                        nc.vector.tensor_copy(u_new[:], ut[:])
                    else:
                        nc.vector.tensor_tensor(u_new[:], u_cur[pr][:], ut[:],
                                                Alu.add)
                    u_cur[pr] = u_new
                    if not last:
                        r_cur[pr] = r_new
                        emit_glue_pair(pr, r_new)

            # ---- epilogue: x = relu(x0 - GS * Ahat^T u'), un-transpose ----
            for c in range(NCH):
                ub = glue_pool.tile([128, W], bf16, tag="ub")
                nc.vector.tensor_copy(
                    ub[:], u_cur[c // 2][:, (c % 2) * W:(c % 2) * W + W])
                xps = mm_psum.tile([128, W], f32, tag="mm")
                for jj in range(CPP):
                    j = c * CPP + jj
                    for nt in range(NT):
                        col = nt * CPP + jj
                        for mt in range(MT):
                            nc.tensor.matmul(
                                xps[:, col:col + 1],
                                ar_sb[:, j, mt, nt * 128:(nt + 1) * 128],
                                ub[:, mt * CPP + jj:mt * CPP + jj + 1],
                                start=(mt == 0), stop=(mt == MT - 1),
                            )
                xsb = glue_pool.tile([128, W], f32, tag="xsb")
                nc.vector.scalar_tensor_tensor(
                    xsb[:], xps[:], -GS, xt_sb[:, c * W:(c + 1) * W],
                    Alu.mult, Alu.add)
                xrel = glue_pool.tile([128, W], f32, tag="xrel")
                nc.vector.tensor_scalar(out=xrel[:], in0=xsb[:], scalar1=0.0,
                                        scalar2=None, op0=Alu.max)
                pT = fin_psum.tile([W, 128], f32, tag="fin")
                nc.tensor.transpose(pT[:], xrel[:], id_sb[:])
                fin = glue_pool.tile([W, 128], f32, tag="fin_sb")
                nc.vector.tensor_copy(fin[:], pT[:])
                for nt in range(NT):
                    nc.sync.dma_start(
                        out=out_d[c * CPP:(c + 1) * CPP,
                                  nt * 128:(nt + 1) * 128],
                        in_=fin[nt * CPP:(nt + 1) * CPP, :],
                    )

    nc.compile()
    return nc


_NC_CACHE = {}


def _get_nc(n_iters=N_ITERS):
    if n_iters not in _NC_CACHE:
        _NC_CACHE[n_iters] = _build_nc(n_iters)
    return _NC_CACHE[n_iters]


def _tcols(v):
    """[P, 512] -> [128, NCH*W] with col = c*W + t*CPP + jj, t = 128-block."""
    return np.ascontiguousarray(
        v.reshape(NCH, CPP, 4, 128).transpose(3, 0, 2, 1).reshape(128, NCH * W))


def _prep_core_inputs(Ac, bc, xc):
    """Ac [P,512,512] f32, bc [P,512], xc [P,512] -> per-core input map."""
    # at[j, p, nt, m] = Ac[j, m, nt*128+p]
    at = np.ascontiguousarray(
        Ac.reshape(P, M, NT, 128).transpose(0, 3, 2, 1)
    ).astype(F8E3)
    # arows[j, p, mt, n] = Ac[j, mt*128+p, n]
    ar = np.ascontiguousarray(
        Ac.reshape(P, MT, 128, N).transpose(0, 2, 1, 3)
    ).astype(F8E3)
    d = (Ac.astype(np.float32) ** 2).sum(axis=2) * GS          # [P, 512]
    eye = np.eye(128, dtype=np.float32)
    return {
        "at": at,
        "arows": ar,
        "bt": _tcols(np.asarray(bc, dtype=np.float32)),
        "x0t": _tcols(np.asarray(xc, dtype=np.float32)),
        "dt": _tcols(d.astype(np.float32)),
        "imask": (1.0 - eye) * GS,
        "ident": eye,
    }


def kernel(x, A, b, var_mask):
    x = np.asarray(x, dtype=np.float32)
    A = np.asarray(A, dtype=np.float32)
    b = np.asarray(b, dtype=np.float32)
    var_mask = np.asarray(var_mask, dtype=np.float32)

    nc = _get_nc()
    in_maps = []
    for c in range(N_CORES):
        bs = slice(c * B_LOC, (c + 1) * B_LOC)
        in_maps.append(
            _prep_core_inputs(
                A[bs].reshape(P, M, N), b[bs].reshape(P, M), x[bs].reshape(P, N)
            )
        )

    res = run_bass_kernel_spmd(nc, in_maps, list(range(N_CORES)))

    out = np.empty((B, S, N), dtype=np.float32)
    for c in range(N_CORES):
        out[c * B_LOC:(c + 1) * B_LOC] = res.results[c]["xout"].reshape(B_LOC, S, N)
    # reference returns x_fin * var_mask (ones per the input spec; keeps the
    # general contract for any mask values)
    out *= var_mask[:, None, :]
    return out
